# revision 13
# baseline (speedup 1.0000x reference)
"""Trainium2 Bass kernel for nn_CumulantSOAP_CV: per-column cumulants of
X (100000, 1024) up to order 5, then (X_cum - mu) @ W -> (1, 8).

Strategy (8 NeuronCores, SPMD):
  - Host casts X to bf16 (same rounding the old device-side DMA cast did)
    and pads to 100352 rows; shards rows across 8 cores: 12544 rows =
    98 tiles of (128, 1024) each. bf16 halves the HBM stream (the f32
    version was DMA-bound at ~2.9us/sub-block).
  - Per core, one pass over X: per 2-tile sub-block compute x^2 (ScalarE
    Square), x^3 = x*x^2 (DVE), x^5 = x^2*x^3 (DVE, with an optional
    column-slice offloaded to GpSimd). Column sums of x, x^2, x^3, x^5
    via ones-vector matmuls accumulated in PSUM at 4 distinct 32-col PE
    positions (concurrent strips). S4 = sum((x^2)^2) via PE "diagonal"
    matmuls x2_chunk^T @ x2_chunk for all 8 chunks, accumulated in PSUM.
  - The cross-core reduction is split in two AllGathers so the expensive
    collective latency overlaps the main loop: plain sums over tiles
    [0, PHASE_A_TILES) are DMA'd straight from PSUM to DRAM mid-loop and
    AllGather'd while the loop continues; the remainder (plus the S4
    diagonal, extracted once at the end) rides a second small AllGather
    at loop end. Each core sums the 16 partial vectors, forms cumulants,
    and does the (1,5120)@(5120,8) projection replicated. Output from
    core 0.
"""

import os

import numpy as np
import ml_dtypes

import concourse.bass as bass
import concourse.mybir as mybir
import concourse.tile as tile
from concourse import bacc
from concourse.bass_utils import run_bass_kernel_spmd
from concourse.masks import make_identity

N_CORES = 8
N_TRUE = 100000
P = 1024
ROWS_PER_CORE = 12544      # 98 tiles of 128
NT = ROWS_PER_CORE // 128  # 98
K_OUT = 8
SCALE = 1.0 / float(N_TRUE)

F32 = mybir.dt.float32
BF16 = mybir.dt.bfloat16
Alu = mybir.AluOpType
Act = mybir.ActivationFunctionType

# columns of x^5 computed on GpSimd instead of DVE (0 disables; >0 measured
# HARMFUL: GpSimd SBUF traffic knocks the DVE cu op from 2x to 1x mode and
# its ~1.9us ops head-of-line-block the PE queue)
GP_COLS = int(os.environ.get("K_GPCOLS", "0"))
# tiles contributing to the early (overlapped) AllGather
PHASE_A_TILES = int(os.environ.get("K_PHASEA", "40"))
NDIAG = 8


def _build(rows_per_core=ROWS_PER_CORE):
    nt_total = rows_per_core // 128
    pa = PHASE_A_TILES
    nc = bacc.Bacc("TRN2", target_bir_lowering=False, debug=False,
                   num_devices=N_CORES)
    X = nc.dram_tensor("X", [rows_per_core, P], BF16, kind="ExternalInput")
    MU = nc.dram_tensor("mu", [1, 5 * P], F32, kind="ExternalInput")
    W = nc.dram_tensor("W", [5 * P, K_OUT], F32, kind="ExternalInput")
    OUT = nc.dram_tensor("out", [1, K_OUT], F32, kind="ExternalOutput")

    # phase A payload: [S1|S2|S3|S5] raw sums over tiles [0, pa)
    ccA_in = nc.dram_tensor("ccA_in", [4 * P], F32)
    ccA_out = nc.dram_tensor("ccA_out", [N_CORES * 4 * P], F32,
                             addr_space="Shared")
    # phase B payload: [S1|S2|S3|S5|S4] (S4 over ALL tiles, rest over
    # tiles [pa, nt_total))
    ccB_in = nc.dram_tensor("ccB_in", [5 * P], F32)
    ccB_out = nc.dram_tensor("ccB_out", [N_CORES * 5 * P], F32,
                             addr_space="Shared")
    warm_in = nc.dram_tensor("warm_in", [5 * P], F32)
    warm_out = nc.dram_tensor("warm_out", [N_CORES * 5 * P], F32,
                              addr_space="Shared")

    # DMA blocks of 4 tiles; compute sub-blocks of 2 tiles
    dbs = [(i, min(4, nt_total - i)) for i in range(0, nt_total, 4)]

    with tile.TileContext(nc) as tc:
        with (
            tc.tile_pool(name="xin", bufs=6) as xin,
            tc.tile_pool(name="pows", bufs=4) as pows,
            tc.tile_pool(name="const", bufs=1) as const,
            tc.tile_pool(name="acc", bufs=1, space="PSUM") as accp,
            tc.tile_pool(name="tailps", bufs=1, space="PSUM") as tailps,
            tc.tile_pool(name="tail", bufs=1) as tailp,
        ):
            Xv = X.ap()

            # issue the first X block's DMA before any constant setup so
            # the HBM stream starts immediately
            t0_0, ndt_0 = dbs[0]
            x_first = xin.tile([128, ndt_0, P], BF16, tag="x")
            nh0 = max(1, ndt_0 // 2)
            for lo in range(0, ndt_0, nh0):
                w = min(nh0, ndt_0 - lo)
                nc.sync.dma_start(
                    out=x_first[:, lo:lo + w, :],
                    in_=Xv[(t0_0 + lo) * 128:(t0_0 + lo + w) * 128, :].rearrange(
                        "(p s) c -> p s c", s=w),
                )

            ones = const.tile([128, 1], BF16)
            nc.vector.memset(ones, 1.0)
            ident = const.tile([128, 128], F32)
            make_identity(nc, ident)
            # identity replicated along free axis for one-shot S4 extract
            ident_rep = const.tile([128, NDIAG, 128], F32)
            for c in range(NDIAG):
                nc.vector.tensor_copy(ident_rep[:, c, :], ident)

            # warm-up collective at the real payload size, launched
            # immediately: absorbs the ncfw/TOPSP cold-start barrier.
            wtile = const.tile([1, 8], F32)
            nc.vector.memset(wtile, 0.0)
            nc.scalar.dma_start(out=warm_in.ap()[0:8], in_=wtile[0:1, :])
            nc.gpsimd.collective_compute(
                "AllGather", Alu.bypass,
                replica_groups=[list(range(N_CORES))],
                ins=[warm_in.ap()],
                outs=[warm_out.ap()],
            )

            # weights / mu for the tail (contraction row j5 = 40*p + q)
            w_s = const.tile([128, 40, K_OUT], F32)
            nc.scalar.dma_start(out=w_s, in_=W.ap().rearrange(
                "(p q) k -> p q k", p=128))
            mu_s = const.tile([128, 40], F32)
            nc.scalar.dma_start(out=mu_s, in_=MU.ap()[0, :].rearrange(
                "(p q) -> p q", p=128))

            # PSUM accumulators, alive across the whole main loop.
            # ps_plainA/B: S1@p0 S2@p32 S3@p64 S5@p96 for the two row
            # phases — separate banks so phase B's start=True matmuls
            # never WAR-stall on phase A's readout. ps_diag accumulates
            # S4 over ALL tiles. 2+2+2 banks (+1 tail) of the 8.
            ps_plainA = accp.tile([128, P], F32)
            ps_plainB = accp.tile([128, P], F32)
            ps_diag = accp.tile([128, NDIAG * 128], F32)

            for bi, (t0, ndt) in enumerate(dbs):
                if bi == 0:
                    x = x_first
                else:
                    x = xin.tile([128, ndt, P], BF16, tag="x")
                    # partition p holds CONSECUTIVE rows -> contiguous
                    # multi-KB DMA runs per partition (row->partition
                    # placement is free for column sums). Two DMAs per
                    # block for finer arrival.
                    nh = max(1, ndt // 2)
                    for lo in range(0, ndt, nh):
                        w = min(nh, ndt - lo)
                        nc.sync.dma_start(
                            out=x[:, lo:lo + w, :],
                            in_=Xv[(t0 + lo) * 128:(t0 + lo + w) * 128, :].rearrange(
                                "(p s) c -> p s c", s=w),
                        )
                # whole 4-tile block in one op per engine: fewer DVE ops
                # amortize the ~360ns per-op overhead (DVE is the binding
                # engine at ~2.4us per 2 tiles)
                sq = pows.tile([128, ndt, P], BF16, tag="sq")
                nc.scalar.activation(sq, x, Act.Square)
                cu = pows.tile([128, ndt, P], BF16, tag="cu")
                nc.vector.tensor_mul(cu, x, sq)
                x5 = pows.tile([128, ndt, P], BF16, tag="x5")
                nc.vector.tensor_mul(x5, sq, cu)

                for t in range(ndt):
                    gt = t0 + t
                    st = gt == 0
                    sp = gt == nt_total - 1
                    # S4 diag blocks x2_chunk^T @ x2_chunk: depend
                    # only on sq; single phase across the whole loop.
                    # start/stop per PSUM bank (chunks 0-3 -> bank 0,
                    # 4-7 -> bank 1).
                    for c in range(NDIAG):
                        cs = slice(c * 128, (c + 1) * 128)
                        nc.tensor.matmul(
                            ps_diag[:, cs], sq[:, t, cs], sq[:, t, cs],
                            start=st and c % 4 == 0,
                            stop=sp and (c == 3 or c == NDIAG - 1),
                            tile_position=(0, 0),
                        )
                    # plain col-sums: 4 powers on 4 concurrent col-strips.
                    # j outer / h inner so consecutive matmuls alternate
                    # PSUM banks (h=0 vs h=1) instead of hammering one.
                    ps_plain = ps_plainA if gt < pa else ps_plainB
                    stp = gt == 0 or gt == pa
                    spp = gt == pa - 1 or gt == nt_total - 1
                    for j, pw in enumerate((x, sq, cu, x5)):
                        bp = 32 * j
                        for h in range(2):
                            sl = slice(h * 512, (h + 1) * 512)
                            nc.tensor.matmul(
                                ps_plain[bp:bp + 1, sl], ones[:, 0:1],
                                pw[:, t, sl],
                                start=stp, stop=spp,
                                tile_position=(0, bp),
                            )

                # phase A readout + early AllGather, emitted two blocks
                # AFTER the boundary so the phase-A stop matmuls are long
                # done and the ACT copy never head-of-line-stalls the sq
                # stream; the collective overlaps the rest of the loop.
                if t0 + ndt == pa + 8:
                    rowsA = tailp.tile([128, P], F32, name="rowsA")
                    nc.scalar.activation(rowsA, ps_plainA, Act.Copy)
                    for jj in range(4):
                        nc.scalar.dma_start(
                            out=ccA_in.ap()[jj * P:(jj + 1) * P],
                            in_=rowsA[32 * jj:32 * jj + 1, :],
                        )
                    nc.gpsimd.collective_compute(
                        "AllGather", Alu.bypass,
                        replica_groups=[list(range(N_CORES))],
                        ins=[ccA_in.ap()],
                        outs=[ccA_out.ap()],
                    )

            # ---- tail ----
            # S4 diagonal extract: (ps_diag * ident_rep) fused with the
            # 128-wide reduce via accum_out, one op per chunk.
            s4_s = tailp.tile([128, NDIAG], F32)
            dummy = tailp.tile([128, 128], F32)
            for c in range(NDIAG):
                nc.vector.scalar_tensor_tensor(
                    dummy, ps_diag[:, c * 128:(c + 1) * 128],
                    1.0, ident_rep[:, c, :], Alu.mult, Alu.mult,
                    accum_out=s4_s[:, c:c + 1])

            # phase B payload: plain rows via one full-tile copy, S4 from SBUF
            rowsB = tailp.tile([128, P], F32, name="rowsB")
            nc.scalar.activation(rowsB, ps_plainB, Act.Copy)
            for jj in range(4):
                eng = nc.scalar if jj % 2 == 0 else nc.sync
                eng.dma_start(
                    out=ccB_in.ap()[jj * P:(jj + 1) * P],
                    in_=rowsB[32 * jj:32 * jj + 1, :],
                )
            nc.sync.dma_start(
                out=ccB_in.ap()[4 * P:5 * P].rearrange("(c i) -> i c", i=128),
                in_=s4_s,
            )
            nc.gpsimd.collective_compute(
                "AllGather", Alu.bypass,
                replica_groups=[list(range(N_CORES))],
                ins=[ccB_in.ap()],
                outs=[ccB_out.ap()],
            )

            # gathered per-core partials -> summed raw moments.
            # momg*[p, k, core, cc]; column j of moment k is (8p + cc).
            # A-side gather + tree-sum depends only on the (long done)
            # phase-A collective, so it overlaps collective B's flight.
            ga = tailp.tile([128, 4, N_CORES, K_OUT], F32)
            gb = tailp.tile([128, 5, N_CORES, K_OUT], F32)
            ccAv = ccA_out.ap().rearrange(
                "(r k p c) -> p k r c", r=N_CORES, k=4, p=128)
            ccBv = ccB_out.ap().rearrange(
                "(r k p c) -> p k r c", r=N_CORES, k=5, p=128)
            for k in range(4):
                eng = nc.sync if k % 2 == 0 else nc.scalar
                eng.dma_start(out=ga[:, k, :, :], in_=ccAv[:, k, :, :])
            nc.vector.tensor_add(ga[:, :, 0:4, :], ga[:, :, 0:4, :],
                                 ga[:, :, 4:8, :])
            nc.vector.tensor_add(ga[:, :, 0:2, :], ga[:, :, 0:2, :],
                                 ga[:, :, 2:4, :])
            nc.vector.tensor_add(ga[:, :, 0:1, :], ga[:, :, 0:1, :],
                                 ga[:, :, 1:2, :])
            for k in range(5):
                eng = nc.scalar if k % 2 == 0 else nc.sync
                eng.dma_start(out=gb[:, k, :, :], in_=ccBv[:, k, :, :])
            nc.vector.tensor_add(gb[:, :, 0:4, :], gb[:, :, 0:4, :],
                                 gb[:, :, 4:8, :])
            nc.vector.tensor_add(gb[:, :, 0:2, :], gb[:, :, 0:2, :],
                                 gb[:, :, 2:4, :])
            nc.vector.tensor_add(gb[:, :, 0:1, :], gb[:, :, 0:1, :],
                                 gb[:, :, 1:2, :])
            # smom[p, k, cc], k in [M1,M2,M3,M5,M4] order; scaled by 1/N
            smom = tailp.tile([128, 5, K_OUT], F32)
            nc.vector.tensor_add(smom[:, 0:4, :], ga[:, :, 0, :],
                                 gb[:, 0:4, 0, :])
            nc.vector.tensor_copy(smom[:, 4, :], gb[:, 4, 0, :])
            nc.vector.tensor_scalar_mul(smom, smom, SCALE)

            m = smom[:, 0, :]
            M2 = smom[:, 1, :]
            M3 = smom[:, 2, :]
            M5 = smom[:, 3, :]
            M4 = smom[:, 4, :]

            stt = nc.vector.scalar_tensor_tensor
            scr = tailp.tile([128, 12, 8], F32)  # scratch (128,8) slots
            m2, m3, m5, a2, a3, a4, mu2, mu3, b1, c3, c4, t1 = (
                scr[:, i, :] for i in range(12))

            # cumulants written straight into interleaved v slices:
            # v[p, 5*cc + k] = c_k(col 8p+cc)
            v = tailp.tile([128, 40], F32)
            vv = v[:].rearrange("p (c k) -> p c k", k=5)

            nc.vector.tensor_mul(m2, m, m)                   # m^2
            nc.vector.tensor_mul(m3, m2, m)                  # m^3
            nc.vector.tensor_mul(m5, m2, m3)                 # m^5
            nc.vector.tensor_sub(mu2, M2, m2)                # mu2 = M2-m^2
            nc.vector.tensor_copy(vv[:, :, 0], m)
            nc.vector.memset(vv[:, :, 1], 0.0)
            nc.vector.tensor_copy(vv[:, :, 2], mu2)
            # mu3 = M3 + (-3 M2)*m + 2 m^3
            stt(b1, M2, -3.0, m, Alu.mult, Alu.mult)         # -3 m M2
            nc.vector.tensor_add(b1, b1, M3)
            stt(mu3, m3, 2.0, b1, Alu.mult, Alu.add)         # +2m^3
            # c3 = mu3 - 3 mu2^2
            stt(c3, mu2, -3.0, mu2, Alu.mult, Alu.mult)
            nc.vector.tensor_add(vv[:, :, 3], c3, mu3)
            # mu5 = M5 - 5 m M4 + 10 m^2 M3 - 10 m^3 M2 + 4 m^5
            stt(a4, M4, -5.0, m, Alu.mult, Alu.mult)
            stt(a3, M3, 10.0, m2, Alu.mult, Alu.mult)
            stt(a2, M2, -10.0, m3, Alu.mult, Alu.mult)
            nc.vector.tensor_add(a4, a4, M5)
            stt(a3, m5, 4.0, a3, Alu.mult, Alu.add)
            nc.vector.tensor_add(a4, a4, a3)
            nc.vector.tensor_add(a4, a4, a2)                 # mu5
            # c4 = mu5 - 10 mu2 mu3
            stt(t1, mu2, -10.0, mu3, Alu.mult, Alu.mult)
            nc.vector.tensor_add(vv[:, :, 4], a4, t1)

            nc.vector.tensor_sub(v, v, mu_s)

            # projection: collapse q on DVE (k-major scratch so one
            # X-axis reduce yields (128, K)), then a single matmul
            # collapses the partition axis.
            wv = w_s[:].rearrange("p q k -> p k q")
            prod = tailp.tile([128, K_OUT, 40], F32)
            for k in range(K_OUT):
                nc.vector.tensor_mul(prod[:, k, :], v, wv[:, k, :])
            colk = tailp.tile([128, K_OUT], F32)
            nc.vector.tensor_reduce(colk, prod,
                                    axis=mybir.AxisListType.X, op=Alu.add)
            ps_out = tailps.tile([1, K_OUT], F32)
            ones_f = tailp.tile([128, 1], F32)
            nc.vector.memset(ones_f, 1.0)
            nc.tensor.matmul(ps_out[0:1, :], ones_f[:, 0:1], colk,
                             start=True, stop=True)
            o_s = tailp.tile([1, K_OUT], F32)
            nc.vector.tensor_copy(o_s, ps_out)
            nc.sync.dma_start(out=OUT.ap(), in_=o_s)

    nc.compile()
    return nc


_NC = None


def _get_nc():
    global _NC
    if _NC is None:
        _NC = _build()
    return _NC


def _shard(X, mu, W):
    Xb = np.asarray(X, dtype=np.float32).astype(ml_dtypes.bfloat16)
    Xp = np.zeros((N_CORES * ROWS_PER_CORE, P), dtype=ml_dtypes.bfloat16)
    Xp[:Xb.shape[0]] = Xb
    return [
        {
            "X": np.ascontiguousarray(Xp[i * ROWS_PER_CORE:(i + 1) * ROWS_PER_CORE]),
            "mu": np.ascontiguousarray(mu.astype(np.float32)),
            "W": np.ascontiguousarray(W.astype(np.float32)),
        }
        for i in range(N_CORES)
    ]


def run(X, mu, W, trace=False, **trace_kwargs):
    nc = _get_nc()
    in_maps = _shard(X, np.asarray(mu), np.asarray(W))
    res = run_bass_kernel_spmd(nc, in_maps, core_ids=list(range(N_CORES)),
                               trace=trace, **trace_kwargs)
    return res


def kernel(X, mu, W):
    res = run(X, mu, W, trace=False)
    return np.asarray(res.results[0]["out"], dtype=np.float32)


# revision 16
# speedup vs baseline: 1.0280x; 1.0280x over previous
"""Trainium2 Bass kernel for nn_CumulantSOAP_CV: per-column cumulants of
X (100000, 1024) up to order 5, then (X_cum - mu) @ W -> (1, 8).

Strategy (8 NeuronCores, SPMD):
  - Host casts X to bf16 (same rounding the old device-side DMA cast did)
    and pads to 100352 rows; shards rows across 8 cores: 12544 rows =
    98 tiles of (128, 1024) each. bf16 halves the HBM stream (the f32
    version was DMA-bound at ~2.9us/sub-block).
  - Per core, one pass over X: per 2-tile sub-block compute x^2 (ScalarE
    Square), x^3 = x*x^2 (DVE), x^5 = x^2*x^3 (DVE, with an optional
    column-slice offloaded to GpSimd). Column sums of x, x^2, x^3, x^5
    via ones-vector matmuls accumulated in PSUM at 4 distinct 32-col PE
    positions (concurrent strips). S4 = sum((x^2)^2) via PE "diagonal"
    matmuls x2_chunk^T @ x2_chunk for all 8 chunks, accumulated in PSUM.
  - The cross-core reduction is split in two AllGathers so the expensive
    collective latency overlaps the main loop: plain sums over tiles
    [0, PHASE_A_TILES) are DMA'd straight from PSUM to DRAM mid-loop and
    AllGather'd while the loop continues; the remainder (plus the S4
    diagonal, extracted once at the end) rides a second small AllGather
    at loop end. Each core sums the 16 partial vectors, forms cumulants,
    and does the (1,5120)@(5120,8) projection replicated. Output from
    core 0.
"""

import os

import numpy as np
import ml_dtypes

import concourse.bass as bass
import concourse.mybir as mybir
import concourse.tile as tile
from concourse import bacc
from concourse.bass_utils import run_bass_kernel_spmd
from concourse.masks import make_identity

N_CORES = 8
N_TRUE = 100000
P = 1024
ROWS_PER_CORE = 12544      # 98 tiles of 128
NT = ROWS_PER_CORE // 128  # 98
K_OUT = 8
SCALE = 1.0 / float(N_TRUE)

F32 = mybir.dt.float32
BF16 = mybir.dt.bfloat16
Alu = mybir.AluOpType
Act = mybir.ActivationFunctionType

# columns of x^5 computed on GpSimd instead of DVE (0 disables; >0 measured
# HARMFUL: GpSimd SBUF traffic knocks the DVE cu op from 2x to 1x mode and
# its ~1.9us ops head-of-line-block the PE queue)
GP_COLS = int(os.environ.get("K_GPCOLS", "0"))
# tiles contributing to the early (overlapped) AllGather
PHASE_A_TILES = int(os.environ.get("K_PHASEA", "40"))
NDIAG = 8


def _build(rows_per_core=ROWS_PER_CORE):
    nt_total = rows_per_core // 128
    pa = PHASE_A_TILES
    nc = bacc.Bacc("TRN2", target_bir_lowering=False, debug=False,
                   num_devices=N_CORES)
    X = nc.dram_tensor("X", [rows_per_core, P], BF16, kind="ExternalInput")
    MU = nc.dram_tensor("mu", [1, 5 * P], F32, kind="ExternalInput")
    W = nc.dram_tensor("W", [5 * P, K_OUT], F32, kind="ExternalInput")
    OUT = nc.dram_tensor("out", [1, K_OUT], F32, kind="ExternalOutput")

    # phase A payload: [S1|S2|S3|S5] raw sums over tiles [0, pa)
    ccA_in = nc.dram_tensor("ccA_in", [4 * P], F32)
    ccA_out = nc.dram_tensor("ccA_out", [N_CORES * 4 * P], F32,
                             addr_space="Shared")
    # phase B payload: [S1|S2|S3|S5|S4] (S4 over ALL tiles, rest over
    # tiles [pa, nt_total))
    ccB_in = nc.dram_tensor("ccB_in", [5 * P], F32)
    ccB_out = nc.dram_tensor("ccB_out", [N_CORES * 5 * P], F32,
                             addr_space="Shared")
    warm_in = nc.dram_tensor("warm_in", [5 * P], F32)
    warm_out = nc.dram_tensor("warm_out", [N_CORES * 5 * P], F32,
                              addr_space="Shared")

    # DMA blocks of 4 tiles; compute sub-blocks of 2 tiles
    dbs = [(i, min(4, nt_total - i)) for i in range(0, nt_total, 4)]

    with tile.TileContext(nc) as tc:
        with (
            tc.tile_pool(name="xin", bufs=6) as xin,
            tc.tile_pool(name="pows", bufs=4) as pows,
            tc.tile_pool(name="const", bufs=1) as const,
            tc.tile_pool(name="acc", bufs=1, space="PSUM") as accp,
            tc.tile_pool(name="tailps", bufs=1, space="PSUM") as tailps,
            tc.tile_pool(name="tail", bufs=1) as tailp,
        ):
            Xv = X.ap()

            # issue the first X block's DMA before any constant setup so
            # the HBM stream starts immediately
            t0_0, ndt_0 = dbs[0]
            x_first = xin.tile([128, ndt_0, P], BF16, tag="x")
            nh0 = max(1, ndt_0 // 2)
            for lo in range(0, ndt_0, nh0):
                w = min(nh0, ndt_0 - lo)
                nc.sync.dma_start(
                    out=x_first[:, lo:lo + w, :],
                    in_=Xv[(t0_0 + lo) * 128:(t0_0 + lo + w) * 128, :].rearrange(
                        "(p s) c -> p s c", s=w),
                )

            ones = const.tile([128, 1], BF16)
            nc.vector.memset(ones, 1.0)
            ident = const.tile([128, 128], F32)
            make_identity(nc, ident)
            # identity replicated along free axis for one-shot S4 extract
            ident_rep = const.tile([128, NDIAG, 128], F32)
            for c in range(NDIAG):
                nc.vector.tensor_copy(ident_rep[:, c, :], ident)

            # warm-up collective at the real payload size, launched
            # immediately: absorbs the ncfw/TOPSP cold-start barrier.
            wtile = const.tile([1, 8], F32)
            nc.vector.memset(wtile, 0.0)
            nc.scalar.dma_start(out=warm_in.ap()[0:8], in_=wtile[0:1, :])
            nc.gpsimd.collective_compute(
                "AllGather", Alu.bypass,
                replica_groups=[list(range(N_CORES))],
                ins=[warm_in.ap()],
                outs=[warm_out.ap()],
            )

            # weights / mu for the tail (contraction row j5 = 40*p + q)
            w_s = const.tile([128, 40, K_OUT], F32)
            nc.scalar.dma_start(out=w_s, in_=W.ap().rearrange(
                "(p q) k -> p q k", p=128))
            mu_s = const.tile([128, 40], F32)
            nc.scalar.dma_start(out=mu_s, in_=MU.ap()[0, :].rearrange(
                "(p q) -> p q", p=128))

            # PSUM accumulators, alive across the whole main loop.
            # ps_plainA/B: S1@p0 S2@p32 S3@p64 S5@p96 for the two row
            # phases — separate banks so phase B's start=True matmuls
            # never WAR-stall on phase A's readout. ps_diag accumulates
            # S4 over ALL tiles. 2+2+2 banks (+1 tail) of the 8.
            ps_plainA = accp.tile([128, P], F32)
            ps_plainB = accp.tile([128, P], F32)
            ps_diag = accp.tile([128, NDIAG * 128], F32)

            for bi, (t0, ndt) in enumerate(dbs):
                if bi == 0:
                    x = x_first
                else:
                    x = xin.tile([128, ndt, P], BF16, tag="x")
                    # partition p holds CONSECUTIVE rows -> contiguous
                    # multi-KB DMA runs per partition (row->partition
                    # placement is free for column sums). Two DMAs per
                    # block for finer arrival.
                    nh = max(1, ndt // 2)
                    for lo in range(0, ndt, nh):
                        w = min(nh, ndt - lo)
                        nc.sync.dma_start(
                            out=x[:, lo:lo + w, :],
                            in_=Xv[(t0 + lo) * 128:(t0 + lo + w) * 128, :].rearrange(
                                "(p s) c -> p s c", s=w),
                        )
                # whole 4-tile block in one op per engine: fewer DVE ops
                # amortize the ~360ns per-op overhead (DVE is the binding
                # engine at ~2.4us per 2 tiles)
                sq = pows.tile([128, ndt, P], BF16, tag="sq")
                nc.scalar.activation(sq, x, Act.Square)
                cu = pows.tile([128, ndt, P], BF16, tag="cu")
                nc.vector.tensor_mul(cu, x, sq)
                x5 = pows.tile([128, ndt, P], BF16, tag="x5")
                nc.vector.tensor_mul(x5, sq, cu)

                for t in range(ndt):
                    gt = t0 + t
                    st = gt == 0
                    sp = gt == nt_total - 1
                    # S4 diag blocks x2_chunk^T @ x2_chunk: depend
                    # only on sq; single phase across the whole loop.
                    # start/stop per PSUM bank (chunks 0-3 -> bank 0,
                    # 4-7 -> bank 1).
                    for c in range(NDIAG):
                        cs = slice(c * 128, (c + 1) * 128)
                        nc.tensor.matmul(
                            ps_diag[:, cs], sq[:, t, cs], sq[:, t, cs],
                            start=st and c % 4 == 0,
                            stop=sp and (c == 3 or c == NDIAG - 1),
                            tile_position=(0, 0),
                        )
                    # plain col-sums: 4 powers on 4 concurrent col-strips
                    # (h outer / j inner: adjacent matmuls sit at distinct
                    # col positions so they overlap in the array)
                    ps_plain = ps_plainA if gt < pa else ps_plainB
                    stp = gt == 0 or gt == pa
                    spp = gt == pa - 1 or gt == nt_total - 1
                    for h in range(2):
                        sl = slice(h * 512, (h + 1) * 512)
                        for j, pw in enumerate((x, sq, cu, x5)):
                            bp = 32 * j
                            nc.tensor.matmul(
                                ps_plain[bp:bp + 1, sl], ones[:, 0:1],
                                pw[:, t, sl],
                                start=stp, stop=spp,
                                tile_position=(0, bp),
                            )

                # phase A readout + early AllGather, emitted two blocks
                # AFTER the boundary so the phase-A stop matmuls are long
                # done and the ACT copy never head-of-line-stalls the sq
                # stream; the collective overlaps the rest of the loop.
                if t0 + ndt == pa + 8:
                    rowsA = tailp.tile([128, P], F32, name="rowsA")
                    nc.scalar.activation(rowsA, ps_plainA, Act.Copy)
                    # payload DMAs on GpSimd SWDGE: separate queues give
                    # parallel completion semaphores (the HWDGE rings
                    # serialize the ~2.8us HBM-write completion per DMA,
                    # which would delay the collective trigger)
                    for jj in range(4):
                        nc.gpsimd.dma_start(
                            out=ccA_in.ap()[jj * P:(jj + 1) * P],
                            in_=rowsA[32 * jj:32 * jj + 1, :],
                        )
                    nc.gpsimd.collective_compute(
                        "AllGather", Alu.bypass,
                        replica_groups=[list(range(N_CORES))],
                        ins=[ccA_in.ap()],
                        outs=[ccA_out.ap()],
                    )

            # ---- tail ----
            # S4 diagonal extract: (ps_diag * ident_rep) fused with the
            # 128-wide reduce via accum_out, one op per chunk.
            s4_s = tailp.tile([128, NDIAG], F32)
            dummy = tailp.tile([128, 128], F32)
            for c in range(NDIAG):
                nc.vector.scalar_tensor_tensor(
                    dummy, ps_diag[:, c * 128:(c + 1) * 128],
                    1.0, ident_rep[:, c, :], Alu.mult, Alu.mult,
                    accum_out=s4_s[:, c:c + 1])

            # phase B payload: plain rows via one full-tile copy, S4 from
            # SBUF. Payload DMAs on GpSimd SWDGE queues so their HBM-write
            # completion semaphores land in parallel (HWDGE rings serialize
            # them at ~2.8us each, delaying the collective trigger).
            rowsB = tailp.tile([128, P], F32, name="rowsB")
            nc.scalar.activation(rowsB, ps_plainB, Act.Copy)
            for jj in range(4):
                nc.gpsimd.dma_start(
                    out=ccB_in.ap()[jj * P:(jj + 1) * P],
                    in_=rowsB[32 * jj:32 * jj + 1, :],
                )
            nc.gpsimd.dma_start(
                out=ccB_in.ap()[4 * P:5 * P].rearrange("(c i) -> i c", i=128),
                in_=s4_s,
            )

            # A-side gather + tree-sum BEFORE the B collective is emitted:
            # it depends only on the (long done) phase-A collective, so it
            # runs during collective B's flight instead of after it.
            ga = tailp.tile([128, 4, N_CORES, K_OUT], F32)
            gb = tailp.tile([128, 5, N_CORES, K_OUT], F32)
            ccAv = ccA_out.ap().rearrange(
                "(r k p c) -> p k r c", r=N_CORES, k=4, p=128)
            ccBv = ccB_out.ap().rearrange(
                "(r k p c) -> p k r c", r=N_CORES, k=5, p=128)
            for k in range(4):
                eng = nc.sync if k % 2 == 0 else nc.scalar
                eng.dma_start(out=ga[:, k, :, :], in_=ccAv[:, k, :, :])
            nc.vector.tensor_add(ga[:, :, 0:4, :], ga[:, :, 0:4, :],
                                 ga[:, :, 4:8, :])
            nc.vector.tensor_add(ga[:, :, 0:2, :], ga[:, :, 0:2, :],
                                 ga[:, :, 2:4, :])
            nc.vector.tensor_add(ga[:, :, 0:1, :], ga[:, :, 0:1, :],
                                 ga[:, :, 1:2, :])

            nc.gpsimd.collective_compute(
                "AllGather", Alu.bypass,
                replica_groups=[list(range(N_CORES))],
                ins=[ccB_in.ap()],
                outs=[ccB_out.ap()],
            )
            for k in range(5):
                eng = nc.scalar if k % 2 == 0 else nc.sync
                eng.dma_start(out=gb[:, k, :, :], in_=ccBv[:, k, :, :])
            nc.vector.tensor_add(gb[:, :, 0:4, :], gb[:, :, 0:4, :],
                                 gb[:, :, 4:8, :])
            nc.vector.tensor_add(gb[:, :, 0:2, :], gb[:, :, 0:2, :],
                                 gb[:, :, 2:4, :])
            nc.vector.tensor_add(gb[:, :, 0:1, :], gb[:, :, 0:1, :],
                                 gb[:, :, 1:2, :])
            # smom[p, k, cc], k in [M1,M2,M3,M5,M4] order; scaled by 1/N
            smom = tailp.tile([128, 5, K_OUT], F32)
            nc.vector.tensor_add(smom[:, 0:4, :], ga[:, :, 0, :],
                                 gb[:, 0:4, 0, :])
            nc.vector.tensor_copy(smom[:, 4, :], gb[:, 4, 0, :])
            nc.vector.tensor_scalar_mul(smom, smom, SCALE)

            m = smom[:, 0, :]
            M2 = smom[:, 1, :]
            M3 = smom[:, 2, :]
            M5 = smom[:, 3, :]
            M4 = smom[:, 4, :]

            stt = nc.vector.scalar_tensor_tensor
            scr = tailp.tile([128, 12, 8], F32)  # scratch (128,8) slots
            m2, m3, m5, a2, a3, a4, mu2, mu3, b1, c3, c4, t1 = (
                scr[:, i, :] for i in range(12))

            # cumulants written straight into interleaved v slices:
            # v[p, 5*cc + k] = c_k(col 8p+cc)
            v = tailp.tile([128, 40], F32)
            vv = v[:].rearrange("p (c k) -> p c k", k=5)

            nc.vector.tensor_mul(m2, m, m)                   # m^2
            nc.vector.tensor_mul(m3, m2, m)                  # m^3
            nc.vector.tensor_mul(m5, m2, m3)                 # m^5
            nc.vector.tensor_sub(mu2, M2, m2)                # mu2 = M2-m^2
            nc.vector.tensor_copy(vv[:, :, 0], m)
            nc.vector.memset(vv[:, :, 1], 0.0)
            nc.vector.tensor_copy(vv[:, :, 2], mu2)
            # mu3 = M3 + (-3 M2)*m + 2 m^3
            stt(b1, M2, -3.0, m, Alu.mult, Alu.mult)         # -3 m M2
            nc.vector.tensor_add(b1, b1, M3)
            stt(mu3, m3, 2.0, b1, Alu.mult, Alu.add)         # +2m^3
            # c3 = mu3 - 3 mu2^2
            stt(c3, mu2, -3.0, mu2, Alu.mult, Alu.mult)
            nc.vector.tensor_add(vv[:, :, 3], c3, mu3)
            # mu5 = M5 - 5 m M4 + 10 m^2 M3 - 10 m^3 M2 + 4 m^5
            stt(a4, M4, -5.0, m, Alu.mult, Alu.mult)
            stt(a3, M3, 10.0, m2, Alu.mult, Alu.mult)
            stt(a2, M2, -10.0, m3, Alu.mult, Alu.mult)
            nc.vector.tensor_add(a4, a4, M5)
            stt(a3, m5, 4.0, a3, Alu.mult, Alu.add)
            nc.vector.tensor_add(a4, a4, a3)
            nc.vector.tensor_add(a4, a4, a2)                 # mu5
            # c4 = mu5 - 10 mu2 mu3
            stt(t1, mu2, -10.0, mu3, Alu.mult, Alu.mult)
            nc.vector.tensor_add(vv[:, :, 4], a4, t1)

            nc.vector.tensor_sub(v, v, mu_s)

            # projection: collapse q on DVE (k-major scratch so one
            # X-axis reduce yields (128, K)), then a single matmul
            # collapses the partition axis.
            wv = w_s[:].rearrange("p q k -> p k q")
            prod = tailp.tile([128, K_OUT, 40], F32)
            for k in range(K_OUT):
                nc.vector.tensor_mul(prod[:, k, :], v, wv[:, k, :])
            colk = tailp.tile([128, K_OUT], F32)
            nc.vector.tensor_reduce(colk, prod,
                                    axis=mybir.AxisListType.X, op=Alu.add)
            ps_out = tailps.tile([1, K_OUT], F32)
            ones_f = tailp.tile([128, 1], F32)
            nc.vector.memset(ones_f, 1.0)
            nc.tensor.matmul(ps_out[0:1, :], ones_f[:, 0:1], colk,
                             start=True, stop=True)
            o_s = tailp.tile([1, K_OUT], F32)
            nc.vector.tensor_copy(o_s, ps_out)
            nc.sync.dma_start(out=OUT.ap(), in_=o_s)

    nc.compile()
    return nc


_NC = None


def _get_nc():
    global _NC
    if _NC is None:
        _NC = _build()
    return _NC


def _shard(X, mu, W):
    Xb = np.asarray(X, dtype=np.float32).astype(ml_dtypes.bfloat16)
    Xp = np.zeros((N_CORES * ROWS_PER_CORE, P), dtype=ml_dtypes.bfloat16)
    Xp[:Xb.shape[0]] = Xb
    return [
        {
            "X": np.ascontiguousarray(Xp[i * ROWS_PER_CORE:(i + 1) * ROWS_PER_CORE]),
            "mu": np.ascontiguousarray(mu.astype(np.float32)),
            "W": np.ascontiguousarray(W.astype(np.float32)),
        }
        for i in range(N_CORES)
    ]


def run(X, mu, W, trace=False, **trace_kwargs):
    nc = _get_nc()
    in_maps = _shard(X, np.asarray(mu), np.asarray(W))
    res = run_bass_kernel_spmd(nc, in_maps, core_ids=list(range(N_CORES)),
                               trace=trace, **trace_kwargs)
    return res


def kernel(X, mu, W):
    res = run(X, mu, W, trace=False)
    return np.asarray(res.results[0]["out"], dtype=np.float32)


# revision 20
# speedup vs baseline: 1.0574x; 1.0286x over previous
"""Trainium2 Bass kernel for nn_CumulantSOAP_CV: per-column cumulants of
X (100000, 1024) up to order 5, then (X_cum - mu) @ W -> (1, 8).

Strategy (8 NeuronCores, SPMD):
  - Host casts X to bf16 (same rounding the old device-side DMA cast did)
    and pads to 100352 rows; shards rows across 8 cores: 12544 rows =
    98 tiles of (128, 1024) each. bf16 halves the HBM stream (the f32
    version was DMA-bound at ~2.9us/sub-block).
  - Per core, one pass over X: per 2-tile sub-block compute x^2 (ScalarE
    Square), x^3 = x*x^2 (DVE), x^5 = x^2*x^3 (DVE, with an optional
    column-slice offloaded to GpSimd). Column sums of x, x^2, x^3, x^5
    via ones-vector matmuls accumulated in PSUM at 4 distinct 32-col PE
    positions (concurrent strips). S4 = sum((x^2)^2) via PE "diagonal"
    matmuls x2_chunk^T @ x2_chunk for all 8 chunks, accumulated in PSUM.
  - The cross-core reduction is split in two AllGathers so the expensive
    collective latency overlaps the main loop: plain sums over tiles
    [0, PHASE_A_TILES) are DMA'd straight from PSUM to DRAM mid-loop and
    AllGather'd while the loop continues; the remainder (plus the S4
    diagonal, extracted once at the end) rides a second small AllGather
    at loop end. Each core sums the 16 partial vectors, forms cumulants,
    and does the (1,5120)@(5120,8) projection replicated. Output from
    core 0.
"""

import os

import numpy as np
import ml_dtypes

import concourse.bass as bass
import concourse.mybir as mybir
import concourse.tile as tile
from concourse import bacc
from concourse.bass_utils import run_bass_kernel_spmd
from concourse.masks import make_identity

N_CORES = 8
N_TRUE = 100000
P = 1024
ROWS_PER_CORE = 12544      # 98 tiles of 128
NT = ROWS_PER_CORE // 128  # 98
K_OUT = 8
SCALE = 1.0 / float(N_TRUE)

F32 = mybir.dt.float32
BF16 = mybir.dt.bfloat16
Alu = mybir.AluOpType
Act = mybir.ActivationFunctionType

# columns of x^5 computed on GpSimd instead of DVE (0 disables; >0 measured
# HARMFUL: GpSimd SBUF traffic knocks the DVE cu op from 2x to 1x mode and
# its ~1.9us ops head-of-line-block the PE queue)
GP_COLS = int(os.environ.get("K_GPCOLS", "0"))
# tiles contributing to the early (overlapped) AllGather
PHASE_A_TILES = int(os.environ.get("K_PHASEA", "40"))
NDIAG = 8


def _build(rows_per_core=ROWS_PER_CORE):
    nt_total = rows_per_core // 128
    pa = PHASE_A_TILES
    nc = bacc.Bacc("TRN2", target_bir_lowering=False, debug=False,
                   num_devices=N_CORES)
    X = nc.dram_tensor("X", [rows_per_core, P], BF16, kind="ExternalInput")
    MU = nc.dram_tensor("mu", [1, 5 * P], F32, kind="ExternalInput")
    W = nc.dram_tensor("W", [5 * P, K_OUT], F32, kind="ExternalInput")
    OUT = nc.dram_tensor("out", [1, K_OUT], F32, kind="ExternalOutput")

    # phase A payload: [S1|S2|S3|S5] raw sums over tiles [0, pa)
    ccA_in = nc.dram_tensor("ccA_in", [4 * P], F32)
    ccA_out = nc.dram_tensor("ccA_out", [N_CORES * 4 * P], F32,
                             addr_space="Shared")
    # phase B payload: [S1|S2|S3|S5|S4] (S4 over ALL tiles, rest over
    # tiles [pa, nt_total))
    ccB_in = nc.dram_tensor("ccB_in", [5 * P], F32)
    ccB_out = nc.dram_tensor("ccB_out", [N_CORES * 5 * P], F32,
                             addr_space="Shared")
    warm_in = nc.dram_tensor("warm_in", [5 * P], F32)
    warm_out = nc.dram_tensor("warm_out", [N_CORES * 5 * P], F32,
                              addr_space="Shared")
    wk_in = nc.dram_tensor("wk_in", [8], BF16)
    wk_out = nc.dram_tensor("wk_out", [N_CORES * 8], BF16,
                            addr_space="Shared")

    # DMA blocks of 4 tiles; compute sub-blocks of 2 tiles
    dbs = [(i, min(4, nt_total - i)) for i in range(0, nt_total, 4)]

    with tile.TileContext(nc) as tc:
        with (
            tc.tile_pool(name="xin", bufs=6) as xin,
            tc.tile_pool(name="pows", bufs=4) as pows,
            tc.tile_pool(name="const", bufs=1) as const,
            tc.tile_pool(name="acc", bufs=1, space="PSUM") as accp,
            tc.tile_pool(name="tailps", bufs=1, space="PSUM") as tailps,
            tc.tile_pool(name="tail", bufs=1) as tailp,
        ):
            Xv = X.ap()

            # issue the first X block's DMA before any constant setup so
            # the HBM stream starts immediately
            t0_0, ndt_0 = dbs[0]
            x_first = xin.tile([128, ndt_0, P], BF16, tag="x")
            nh0 = max(1, ndt_0 // 2)
            for lo in range(0, ndt_0, nh0):
                w = min(nh0, ndt_0 - lo)
                nc.sync.dma_start(
                    out=x_first[:, lo:lo + w, :],
                    in_=Xv[(t0_0 + lo) * 128:(t0_0 + lo + w) * 128, :].rearrange(
                        "(p s) c -> p s c", s=w),
                )

            ones = const.tile([128, 1], BF16)
            nc.vector.memset(ones, 1.0)
            # identity + tail constants on GpSimd (idle during the loop)
            # so the DVE/ACT startup path stays clear
            ident = const.tile([128, 128], F32)
            make_identity(nc, ident)
            # identity replicated along free axis for one-shot S4 extract
            ident_rep = const.tile([128, NDIAG, 128], F32)
            for c in range(NDIAG):
                nc.gpsimd.tensor_copy(ident_rep[:, c, :], ident)

            # warm-up collective at the real payload size, launched
            # immediately: absorbs the ncfw/TOPSP cold-start barrier.
            wtile = const.tile([1, 8], F32)
            nc.vector.memset(wtile, 0.0)
            nc.gpsimd.dma_start(out=warm_in.ap()[0:8], in_=wtile[0:1, :])
            nc.gpsimd.collective_compute(
                "AllGather", Alu.bypass,
                replica_groups=[list(range(N_CORES))],
                ins=[warm_in.ap()],
                outs=[warm_out.ap()],
            )

            # PSUM accumulators, alive across the whole main loop.
            # ps_plainA/B: S1@p0 S2@p32 S3@p64 S5@p96 for the two row
            # phases — separate banks so phase B's start=True matmuls
            # never WAR-stall on phase A's readout. ps_diag accumulates
            # S4 over ALL tiles. 2+2+2 banks (+1 tail) of the 8.
            ps_plainA = accp.tile([128, P], F32)
            ps_plainB = accp.tile([128, P], F32)
            ps_diag = accp.tile([128, NDIAG * 128], F32)

            for bi, (t0, ndt) in enumerate(dbs):
                if bi == 0:
                    x = x_first
                else:
                    x = xin.tile([128, ndt, P], BF16, tag="x")
                    # partition p holds CONSECUTIVE rows -> contiguous
                    # multi-KB DMA runs per partition (row->partition
                    # placement is free for column sums). Two DMAs per
                    # block for finer arrival.
                    nh = max(1, ndt // 2)
                    for lo in range(0, ndt, nh):
                        w = min(nh, ndt - lo)
                        nc.sync.dma_start(
                            out=x[:, lo:lo + w, :],
                            in_=Xv[(t0 + lo) * 128:(t0 + lo + w) * 128, :].rearrange(
                                "(p s) c -> p s c", s=w),
                        )
                # whole 4-tile block in one op per engine: fewer DVE ops
                # amortize the ~360ns per-op overhead (DVE is the binding
                # engine at ~2.4us per 2 tiles)
                sq = pows.tile([128, ndt, P], BF16, tag="sq")
                nc.scalar.activation(sq, x, Act.Square)
                cu = pows.tile([128, ndt, P], BF16, tag="cu")
                nc.vector.tensor_mul(cu, x, sq)
                x5 = pows.tile([128, ndt, P], BF16, tag="x5")
                nc.vector.tensor_mul(x5, sq, cu)

                for t in range(ndt):
                    gt = t0 + t
                    st = gt == 0
                    sp = gt == nt_total - 1
                    # S4 diag blocks x2_chunk^T @ x2_chunk: depend
                    # only on sq; single phase across the whole loop.
                    # start/stop per PSUM bank (chunks 0-3 -> bank 0,
                    # 4-7 -> bank 1).
                    for c in range(NDIAG):
                        cs = slice(c * 128, (c + 1) * 128)
                        nc.tensor.matmul(
                            ps_diag[:, cs], sq[:, t, cs], sq[:, t, cs],
                            start=st and c % 4 == 0,
                            stop=sp and (c == 3 or c == NDIAG - 1),
                            tile_position=(0, 0),
                        )
                    # plain col-sums: 4 powers on 4 concurrent col-strips
                    # (h outer / j inner: adjacent matmuls sit at distinct
                    # col positions so they overlap in the array)
                    ps_plain = ps_plainA if gt < pa else ps_plainB
                    stp = gt == 0 or gt == pa
                    spp = gt == pa - 1 or gt == nt_total - 1
                    for h in range(2):
                        sl = slice(h * 512, (h + 1) * 512)
                        for j, pw in enumerate((x, sq, cu, x5)):
                            bp = 32 * j
                            nc.tensor.matmul(
                                ps_plain[bp:bp + 1, sl], ones[:, 0:1],
                                pw[:, t, sl],
                                start=stp, stop=spp,
                                tile_position=(0, bp),
                            )

                # phase A readout + early AllGather, emitted two blocks
                # AFTER the boundary so the phase-A stop matmuls are long
                # done and the ACT copy never head-of-line-stalls the sq
                # stream; the collective overlaps the rest of the loop.
                if t0 + ndt == pa + 8:
                    rowsA = tailp.tile([128, P], F32, name="rowsA")
                    nc.scalar.activation(rowsA, ps_plainA, Act.Copy)
                    # payload DMAs on GpSimd SWDGE: separate queues give
                    # parallel completion semaphores (the HWDGE rings
                    # serialize the ~2.8us HBM-write completion per DMA,
                    # which would delay the collective trigger)
                    for jj in range(4):
                        nc.gpsimd.dma_start(
                            out=ccA_in.ap()[jj * P:(jj + 1) * P],
                            in_=rowsA[32 * jj:32 * jj + 1, :],
                        )
                    nc.gpsimd.collective_compute(
                        "AllGather", Alu.bypass,
                        replica_groups=[list(range(N_CORES))],
                        ins=[ccA_in.ap()],
                        outs=[ccA_out.ap()],
                    )

                # keep-warm gate: tiny AllGather gated on a late block's
                # x5 tile so ncfw is still awake when the real phase-B
                # collective triggers (it re-cools during long idle gaps,
                # costing ~8us of wakeup + entry barrier at the tail).
                if t0 + ndt == 88:
                    nc.gpsimd.dma_start(out=wk_in.ap(), in_=x5[0:1, 0, 0:8])
                    nc.gpsimd.collective_compute(
                        "AllGather", Alu.bypass,
                        replica_groups=[list(range(N_CORES))],
                        ins=[wk_in.ap()],
                        outs=[wk_out.ap()],
                    )

            # ---- tail ----
            # S4 diagonal extract: (ps_diag * ident_rep) fused with the
            # 128-wide reduce via accum_out, one op per chunk.
            s4_s = tailp.tile([128, NDIAG], F32)
            dummy = tailp.tile([128, 128], F32)
            for c in range(NDIAG):
                nc.vector.scalar_tensor_tensor(
                    dummy, ps_diag[:, c * 128:(c + 1) * 128],
                    1.0, ident_rep[:, c, :], Alu.mult, Alu.mult,
                    accum_out=s4_s[:, c:c + 1])

            # phase B payload: plain rows via one full-tile copy, S4 from
            # SBUF. Payload DMAs on GpSimd SWDGE queues so their HBM-write
            # completion semaphores land in parallel (HWDGE rings serialize
            # them at ~2.8us each, delaying the collective trigger).
            rowsB = tailp.tile([128, P], F32, name="rowsB")
            nc.scalar.activation(rowsB, ps_plainB, Act.Copy)
            for jj in range(4):
                nc.gpsimd.dma_start(
                    out=ccB_in.ap()[jj * P:(jj + 1) * P],
                    in_=rowsB[32 * jj:32 * jj + 1, :],
                )
            nc.gpsimd.dma_start(
                out=ccB_in.ap()[4 * P:5 * P].rearrange("(c i) -> i c", i=128),
                in_=s4_s,
            )

            nc.gpsimd.collective_compute(
                "AllGather", Alu.bypass,
                replica_groups=[list(range(N_CORES))],
                ins=[ccB_in.ap()],
                outs=[ccB_out.ap()],
            )

            # A-side gather + tree-sum entirely on GpSimd, emitted right
            # after the B trigger: depends only on the (long done) phase-A
            # collective, so it runs during collective B's flight on an
            # engine no other tail work needs.
            ga = tailp.tile([128, 4, N_CORES, K_OUT], F32)
            gb = tailp.tile([128, 5, N_CORES, K_OUT], F32)
            ccAv = ccA_out.ap().rearrange(
                "(r k p c) -> p k r c", r=N_CORES, k=4, p=128)
            ccBv = ccB_out.ap().rearrange(
                "(r k p c) -> p k r c", r=N_CORES, k=5, p=128)
            for k in range(4):
                nc.gpsimd.dma_start(out=ga[:, k, :, :], in_=ccAv[:, k, :, :])
            nc.gpsimd.tensor_add(ga[:, :, 0:4, :], ga[:, :, 0:4, :],
                                 ga[:, :, 4:8, :])
            nc.gpsimd.tensor_add(ga[:, :, 0:2, :], ga[:, :, 0:2, :],
                                 ga[:, :, 2:4, :])
            nc.gpsimd.tensor_add(ga[:, :, 0:1, :], ga[:, :, 0:1, :],
                                 ga[:, :, 1:2, :])

            # weights / mu for the projection (contraction row j5 = 40p+q)
            w_s = const.tile([128, 40, K_OUT], F32)
            nc.scalar.dma_start(out=w_s, in_=W.ap().rearrange(
                "(p q) k -> p q k", p=128))
            mu_s = const.tile([128, 40], F32)
            nc.scalar.dma_start(out=mu_s, in_=MU.ap()[0, :].rearrange(
                "(p q) -> p q", p=128))

            for k in range(5):
                eng = nc.scalar if k % 2 == 0 else nc.sync
                eng.dma_start(out=gb[:, k, :, :], in_=ccBv[:, k, :, :])
            nc.vector.tensor_add(gb[:, :, 0:4, :], gb[:, :, 0:4, :],
                                 gb[:, :, 4:8, :])
            nc.vector.tensor_add(gb[:, :, 0:2, :], gb[:, :, 0:2, :],
                                 gb[:, :, 2:4, :])
            nc.vector.tensor_add(gb[:, :, 0:1, :], gb[:, :, 0:1, :],
                                 gb[:, :, 1:2, :])
            # smom[p, k, cc], k in [M1,M2,M3,M5,M4] order; scaled by 1/N
            smom = tailp.tile([128, 5, K_OUT], F32)
            nc.vector.tensor_add(smom[:, 0:4, :], ga[:, :, 0, :],
                                 gb[:, 0:4, 0, :])
            nc.vector.tensor_copy(smom[:, 4, :], gb[:, 4, 0, :])
            nc.vector.tensor_scalar_mul(smom, smom, SCALE)

            m = smom[:, 0, :]
            M2 = smom[:, 1, :]
            M3 = smom[:, 2, :]
            M5 = smom[:, 3, :]
            M4 = smom[:, 4, :]

            stt = nc.vector.scalar_tensor_tensor
            scr = tailp.tile([128, 12, 8], F32)  # scratch (128,8) slots
            m2, m3, m5, a2, a3, a4, mu2, mu3, b1, c3, c4, t1 = (
                scr[:, i, :] for i in range(12))

            # cumulants written straight into interleaved v slices:
            # v[p, 5*cc + k] = c_k(col 8p+cc)
            v = tailp.tile([128, 40], F32)
            vv = v[:].rearrange("p (c k) -> p c k", k=5)

            nc.vector.tensor_mul(m2, m, m)                   # m^2
            nc.vector.tensor_mul(m3, m2, m)                  # m^3
            nc.vector.tensor_mul(m5, m2, m3)                 # m^5
            nc.vector.tensor_sub(mu2, M2, m2)                # mu2 = M2-m^2
            nc.vector.tensor_copy(vv[:, :, 0], m)
            nc.vector.memset(vv[:, :, 1], 0.0)
            nc.vector.tensor_copy(vv[:, :, 2], mu2)
            # mu3 = M3 + (-3 M2)*m + 2 m^3
            stt(b1, M2, -3.0, m, Alu.mult, Alu.mult)         # -3 m M2
            nc.vector.tensor_add(b1, b1, M3)
            stt(mu3, m3, 2.0, b1, Alu.mult, Alu.add)         # +2m^3
            # c3 = mu3 - 3 mu2^2
            stt(c3, mu2, -3.0, mu2, Alu.mult, Alu.mult)
            nc.vector.tensor_add(vv[:, :, 3], c3, mu3)
            # mu5 = M5 - 5 m M4 + 10 m^2 M3 - 10 m^3 M2 + 4 m^5
            stt(a4, M4, -5.0, m, Alu.mult, Alu.mult)
            stt(a3, M3, 10.0, m2, Alu.mult, Alu.mult)
            stt(a2, M2, -10.0, m3, Alu.mult, Alu.mult)
            nc.vector.tensor_add(a4, a4, M5)
            stt(a3, m5, 4.0, a3, Alu.mult, Alu.add)
            nc.vector.tensor_add(a4, a4, a3)
            nc.vector.tensor_add(a4, a4, a2)                 # mu5
            # c4 = mu5 - 10 mu2 mu3
            stt(t1, mu2, -10.0, mu3, Alu.mult, Alu.mult)
            nc.vector.tensor_add(vv[:, :, 4], a4, t1)

            nc.vector.tensor_sub(v, v, mu_s)

            # projection: collapse q on DVE (k-major scratch so one
            # X-axis reduce yields (128, K)), then a single matmul
            # collapses the partition axis.
            wv = w_s[:].rearrange("p q k -> p k q")
            prod = tailp.tile([128, K_OUT, 40], F32)
            for k in range(K_OUT):
                nc.vector.tensor_mul(prod[:, k, :], v, wv[:, k, :])
            colk = tailp.tile([128, K_OUT], F32)
            nc.vector.tensor_reduce(colk, prod,
                                    axis=mybir.AxisListType.X, op=Alu.add)
            ps_out = tailps.tile([1, K_OUT], F32)
            ones_f = tailp.tile([128, 1], F32)
            nc.vector.memset(ones_f, 1.0)
            nc.tensor.matmul(ps_out[0:1, :], ones_f[:, 0:1], colk,
                             start=True, stop=True)
            o_s = tailp.tile([1, K_OUT], F32)
            nc.vector.tensor_copy(o_s, ps_out)
            nc.sync.dma_start(out=OUT.ap(), in_=o_s)

    nc.compile()
    return nc


_NC = None


def _get_nc():
    global _NC
    if _NC is None:
        _NC = _build()
    return _NC


def _shard(X, mu, W):
    Xb = np.asarray(X, dtype=np.float32).astype(ml_dtypes.bfloat16)
    Xp = np.zeros((N_CORES * ROWS_PER_CORE, P), dtype=ml_dtypes.bfloat16)
    Xp[:Xb.shape[0]] = Xb
    return [
        {
            "X": np.ascontiguousarray(Xp[i * ROWS_PER_CORE:(i + 1) * ROWS_PER_CORE]),
            "mu": np.ascontiguousarray(mu.astype(np.float32)),
            "W": np.ascontiguousarray(W.astype(np.float32)),
        }
        for i in range(N_CORES)
    ]


def run(X, mu, W, trace=False, **trace_kwargs):
    nc = _get_nc()
    in_maps = _shard(X, np.asarray(mu), np.asarray(W))
    res = run_bass_kernel_spmd(nc, in_maps, core_ids=list(range(N_CORES)),
                               trace=trace, **trace_kwargs)
    return res


def kernel(X, mu, W):
    res = run(X, mu, W, trace=False)
    return np.asarray(res.results[0]["out"], dtype=np.float32)
